# revision 9
# baseline (speedup 1.0000x reference)
"""Multi-head attention (B=4, N=2048, D=1024, H=16) on 8 Trainium2 NeuronCores.

Sharding: core = (batch b = core//2, head-group g = core%2 of 8 heads).
Each core computes qkv + attention for its 8 heads and a *partial* output
projection over its 512 features; the host sums the two partials per batch
and adds the bias (the tensor-parallel unshard).

All matmuls run in fp32r (TF32-like, full PE speed at moving dim 512).
Scores are computed transposed (S^T[m,n]: keys on partitions) so softmax
needs no on-chip transpose; a ones-column appended to v yields the softmax
denominators inside the same PE accumulation as attn@v.
"""
import sys

sys.path.insert(0, '/opt/trn_rl_repo')

import numpy as np

import concourse.bass as bass  # noqa: F401  (registers engines)
import concourse.mybir as mybir
import concourse.tile as tile
from concourse import bacc
from concourse.bass_utils import run_bass_kernel_spmd

dt = mybir.dt

B = 4
N = 2048          # sequence length
D = 1024          # d_model
NH = 16           # total heads
HD = 64           # head dim
NHC = 8           # heads per core
DC = NHC * HD     # 512 features per core
SCALE = HD ** -0.5

P = 128           # partitions
KB = D // P       # 8 k-blocks
NCH = N // 512    # 4 n-chunks of 512
MT = N // P       # 16 m-tiles of 128
DB = DC // P      # 4 d'-blocks / c-blocks


def build_program(debug=False):
    nc = bacc.Bacc("TRN2", target_bir_lowering=False, debug=False,
                   enable_asserts=False, num_devices=8)

    xT = nc.dram_tensor("xT", [D, N], dt.float32, kind="ExternalInput")
    wqT = nc.dram_tensor("wqT", [D, DC], dt.float32, kind="ExternalInput")
    wkT = nc.dram_tensor("wkT", [D, DC], dt.float32, kind="ExternalInput")
    wvT = nc.dram_tensor("wvT", [D, DC], dt.float32, kind="ExternalInput")
    wpT = nc.dram_tensor("wpT", [DC, D], dt.float32, kind="ExternalInput")
    out = nc.dram_tensor("out", [N, D], dt.float32, kind="ExternalOutput")
    if debug:
        dbg_qT = nc.dram_tensor("dbg_qT", [P, DB, N], dt.float32, kind="ExternalOutput")
        dbg_kT = nc.dram_tensor("dbg_kT", [P, DB, N], dt.float32, kind="ExternalOutput")
        dbg_v = nc.dram_tensor("dbg_v", [P, MT, NHC, HD + 1], dt.float32, kind="ExternalOutput")
        dbg_e = nc.dram_tensor("dbg_e", [P, 1024], dt.float32, kind="ExternalOutput")
        dbg_at = nc.dram_tensor("dbg_at", [P, DB, 512], dt.float32, kind="ExternalOutput")
        dbg_po = nc.dram_tensor("dbg_po", [P, 512], dt.float32, kind="ExternalOutput")
        dbg_bc = nc.dram_tensor("dbg_bc", [HD, 512], dt.float32, kind="ExternalOutput")

    f32r = dt.float32r
    Exp = mybir.ActivationFunctionType.Exp

    with tile.TileContext(nc) as tc:
        with tc.tile_pool(name="persist", bufs=1) as persist, \
             tc.tile_pool(name="small", bufs=2) as small_pool, \
             tc.tile_pool(name="outsb", bufs=3) as out_pool, \
             tc.tile_pool(name="ps_mm", bufs=2, space="PSUM") as ps_mm, \
             tc.tile_pool(name="ps_S", bufs=1, space="PSUM") as ps_S, \
             tc.tile_pool(name="ps_o", bufs=1, space="PSUM") as ps_o:

            # ---- persistent SBUF tensors ----
            wp_sb = persist.tile([P, DB, D], f32r, tag="wp")
            qT_sb = persist.tile([P, DB, N], f32r, tag="qT")
            kT_sb = persist.tile([P, DB, N], f32r, tag="kT")
            # v with a ones column per head: [m-part, m-tile, head, 65]
            v_sb = persist.tile([P, MT, NHC, HD + 1], f32r, tag="v")
            ones_sb = persist.tile([P, HD], f32r, tag="ones")

            nc.sync.dma_start(
                wp_sb[:], wpT.ap().rearrange("(cb p) e -> p cb e", p=P).bitcast(f32r))
            nc.vector.memset(v_sb[:].bitcast(dt.float32), 1.0)
            nc.vector.memset(ones_sb[:].bitcast(dt.float32), 1.0)

            # ---- phase 1: qkv projections (weights scoped to this phase) ----
            qkv_scope = tc.tile_pool(name="wqkv", bufs=1)
            wq_pool = qkv_scope.__enter__()
            xw_scope = tc.tile_pool(name="xw", bufs=1)
            xw_pool = xw_scope.__enter__()

            wq_sb = wq_pool.tile([P, KB, DC], f32r, tag="wq")
            wk_sb = wq_pool.tile([P, KB, DC], f32r, tag="wk")
            wv_sb = wq_pool.tile([P, KB, DC], f32r, tag="wv")
            nc.sync.dma_start(
                wq_sb[:], wqT.ap().rearrange("(kb p) d -> p kb d", p=P).bitcast(f32r))
            nc.sync.dma_start(
                wk_sb[:], wkT.ap().rearrange("(kb p) d -> p kb d", p=P).bitcast(f32r))
            nc.sync.dma_start(
                wv_sb[:], wvT.ap().rearrange("(kb p) d -> p kb d", p=P).bitcast(f32r))

            for j in range(NCH):
                nsl = slice(j * 512, (j + 1) * 512)
                xw = xw_pool.tile([P, KB, 512], f32r, tag="xw")
                nc.sync.dma_start(
                    xw[:],
                    xT.ap()[:, nsl].rearrange("(kb p) n -> p kb n", p=P).bitcast(f32r))

                for dst_sb, w_sb in ((kT_sb, wk_sb), (qT_sb, wq_sb)):
                    for db in range(DB):
                        pq = ps_mm.tile([P, 512], dt.float32, tag="mm")
                        for kb in range(KB):
                            nc.tensor.matmul(
                                pq[:],
                                lhsT=w_sb[:, kb, db * P:(db + 1) * P],
                                rhs=xw[:, kb, :],
                                start=(kb == 0), stop=(kb == KB - 1))
                        nc.vector.tensor_copy(out=dst_sb[:, db, nsl], in_=pq[:])

                for mc in range(4):
                    m = j * 4 + mc
                    pv = ps_mm.tile([P, 512], dt.float32, tag="mm")
                    for kb in range(KB):
                        nc.tensor.matmul(
                            pv[:],
                            lhsT=xw[:, kb, mc * P:(mc + 1) * P],
                            rhs=wv_sb[:, kb, :],
                            start=(kb == 0), stop=(kb == KB - 1))
                    nc.vector.tensor_copy(
                        out=v_sb[:, m, :, 0:HD],
                        in_=pv[:].rearrange("p (h d) -> p h d", h=NHC))

            xw_scope.__exit__(None, None, None)
            qkv_scope.__exit__(None, None, None)

            if debug:
                nc.sync.dma_start(dbg_qT.ap(), qT_sb[:].bitcast(dt.float32))
                nc.sync.dma_start(dbg_kT.ap(), kT_sb[:].bitcast(dt.float32))
                nc.sync.dma_start(dbg_v.ap(), v_sb[:].bitcast(dt.float32))

            # ---- phase 2+3: attention + projection, per n-chunk ----
            expS_scope = tc.tile_pool(name="expS", bufs=2)
            expS_pool = expS_scope.__enter__()
            at_scope = tc.tile_pool(name="at", bufs=2)
            at_pool = at_scope.__enter__()

            for j in range(NCH):
                nsl = slice(j * 512, (j + 1) * 512)
                at = at_pool.tile([P, DB, 512], f32r, tag="at")

                for p in range(DB):  # head pair p -> heads 2p, 2p+1
                    po = [ps_o.tile([P, 512], dt.float32, tag=f"o{h}", name=f"po{h}")
                          for h in range(2)]

                    for mp in range(MT // 2):
                        S = [ps_S.tile([P, 1024], dt.float32, tag=f"S{h}", name=f"S{h}")
                             for h in range(2)]
                        for half in range(2):
                            m = 2 * mp + half
                            msl = slice(m * P, (m + 1) * P)
                            hsl = slice(half * 512, (half + 1) * 512)
                            for h in range(2):
                                rsl = slice(h * HD, (h + 1) * HD)
                                nc.tensor.matmul(
                                    S[h][:, hsl],
                                    lhsT=kT_sb[rsl, p, msl],
                                    rhs=qT_sb[rsl, p, nsl],
                                    start=True, stop=True)
                        eS = [expS_pool.tile([P, 1024], f32r, tag=f"e{h}", name=f"eS{h}")
                              for h in range(2)]
                        for h in range(2):
                            nc.scalar.activation(eS[h][:], S[h][:], Exp, scale=SCALE)
                        if debug and j == 0 and p == 0 and mp == 0:
                            nc.sync.dma_start(dbg_e.ap(), eS[0][:].bitcast(dt.float32))
                        for half in range(2):
                            m = 2 * mp + half
                            hsl = slice(half * 512, (half + 1) * 512)
                            for h in range(2):
                                nc.tensor.matmul(
                                    po[h][0:HD + 1, :],
                                    lhsT=v_sb[:, m, 2 * p + h, :],
                                    rhs=eS[h][:, hsl],
                                    start=(m == 0), stop=(m == MT - 1))

                    if debug and j == 0 and p == 0:
                        pod = out_pool.tile([P, 512], dt.float32, tag="osb", name="pod")
                        nc.vector.tensor_copy(out=pod[:], in_=po[0][:])
                        nc.sync.dma_start(dbg_po.ap(), pod[:])

                    # epilogue: normalize and place into at[c-block p].
                    # Broadcast sums across partitions via a K=1 matmul
                    # (ones[1,HD]^T @ sums[1,512]), then reciprocal on DVE.
                    for h in range(2):
                        sums_r = small_pool.tile([HD + 1, 512], f32r, tag="rc")
                        nc.vector.tensor_copy(out=sums_r[HD:HD + 1, :],
                                              in_=po[h][HD:HD + 1, :])
                        bcp = ps_S.tile([P, 1024], dt.float32, tag="S0",
                                        name=f"bcp{h}")
                        nc.tensor.matmul(bcp[0:HD, 0:512],
                                         lhsT=ones_sb[HD:HD + 1, :],
                                         rhs=sums_r[HD:HD + 1, :],
                                         start=True, stop=True)
                        bc = small_pool.tile([HD, 512], dt.float32, tag="bc")
                        nc.vector.reciprocal(bc[:], bcp[0:HD, 0:512])
                        if debug and j == 0 and p == 0 and h == 0:
                            nc.sync.dma_start(dbg_bc.ap(), bc[:])
                        if h == 0:
                            nc.vector.tensor_tensor(
                                out=at[0:HD, p, :], in0=po[h][0:HD, :], in1=bc[:],
                                op=mybir.AluOpType.mult)
                        else:
                            tmp = small_pool.tile([HD, 512], f32r, tag="tmp")
                            nc.vector.tensor_tensor(
                                out=tmp[:], in0=po[h][0:HD, :], in1=bc[:],
                                op=mybir.AluOpType.mult)
                            nc.sync.dma_start(at[HD:P, p, :], tmp[:])

                if debug and j == 0:
                    nc.sync.dma_start(dbg_at.ap(), at[:].bitcast(dt.float32))

                # projection for this n-chunk (partial over this core's 512 c)
                for ns in range(4):
                    rsl = slice(j * 512 + ns * P, j * 512 + (ns + 1) * P)
                    for ec in range(2):
                        esl = slice(ec * 512, (ec + 1) * 512)
                        pp = ps_mm.tile([P, 512], dt.float32, tag="mm")
                        for cb in range(DB):
                            nc.tensor.matmul(
                                pp[:],
                                lhsT=at[:, cb, ns * P:(ns + 1) * P],
                                rhs=wp_sb[:, cb, esl],
                                start=(cb == 0), stop=(cb == DB - 1))
                        osb = out_pool.tile([P, 512], dt.float32, tag="osb")
                        nc.vector.tensor_copy(out=osb[:], in_=pp[:])
                        nc.sync.dma_start(out.ap()[rsl, esl], osb[:])

            at_scope.__exit__(None, None, None)
            expS_scope.__exit__(None, None, None)

    nc.compile()
    return nc


_CACHE: dict = {}


def _get_program():
    if "nc" not in _CACHE:
        _CACHE["nc"] = build_program()
    return _CACHE["nc"]


def make_in_maps(x, w_qkv, w_proj):
    """Host-side sharding: per-core input dict."""
    x = np.ascontiguousarray(np.asarray(x, dtype=np.float32))
    w_qkv = np.asarray(w_qkv, dtype=np.float32)
    w_proj = np.asarray(w_proj, dtype=np.float32)
    in_maps = []
    for core in range(8):
        b, g = divmod(core, 2)
        gsl = slice(g * DC, (g + 1) * DC)
        in_maps.append({
            "xT": np.ascontiguousarray(x[b].T),                       # [D, N]
            "wqT": np.ascontiguousarray(w_qkv[0 * D:1 * D][gsl].T),   # [D, DC]
            "wkT": np.ascontiguousarray(w_qkv[1 * D:2 * D][gsl].T),
            "wvT": np.ascontiguousarray(w_qkv[2 * D:3 * D][gsl].T),
            "wpT": np.ascontiguousarray(w_proj[:, gsl].T),            # [DC, D]
        })
    return in_maps


def run(x, w_qkv, w_proj, b_proj, **spmd_kwargs):
    nc = _get_program()
    in_maps = make_in_maps(x, w_qkv, w_proj)
    res = run_bass_kernel_spmd(nc, in_maps, list(range(8)), **spmd_kwargs)
    b_proj = np.asarray(b_proj, dtype=np.float32)
    outp = np.empty((B, N, D), dtype=np.float32)
    for b in range(B):
        outp[b] = (res.results[2 * b]["out"] + res.results[2 * b + 1]["out"]
                   + b_proj[None, :])
    return outp, res


def kernel(x, w_qkv, w_proj, b_proj):
    outp, _ = run(x, w_qkv, w_proj, b_proj)
    return outp


# revision 12
# speedup vs baseline: 1.0151x; 1.0151x over previous
"""Multi-head attention (B=4, N=2048, D=1024, H=16) on 8 Trainium2 NeuronCores.

Sharding: core = (batch b = core//2, head-group g = core%2 of 8 heads).
Each core computes qkv + attention for its 8 heads and a *partial* output
projection over its 512 features; the host sums the two partials per batch
and adds the bias (the tensor-parallel unshard).

All matmuls run in fp32r (TF32-like, full PE speed at moving dim >=256).
Scores are computed transposed (S^T[m,n]: keys on partitions) so softmax
needs no on-chip transpose; a ones-column appended to v yields the softmax
denominators inside the same PE accumulation as attn@v.

Emission is software-pipelined for the ACT engine (exp is the per-core
roofline: 33.5M elements at 1 elem/lane/cycle): a short prelude computes
kT, v and qT(chunk 0); per m-pair the attn@v matmuls of iteration i-1 are
emitted between the score matmuls so the PE never blocks the next exp, and
qT for chunk j+1 is produced during chunk j's attention.
"""
import sys

sys.path.insert(0, '/opt/trn_rl_repo')

import numpy as np

import concourse.bass as bass  # noqa: F401  (registers engines)
import concourse.mybir as mybir
import concourse.tile as tile
from concourse import bacc
from concourse.bass_utils import run_bass_kernel_spmd

dt = mybir.dt

B = 4
N = 2048          # sequence length
D = 1024          # d_model
NH = 16           # total heads
HD = 64           # head dim
NHC = 8           # heads per core
DC = NHC * HD     # 512 features per core
SCALE = HD ** -0.5

P = 128           # partitions
KB = D // P       # 8 k-blocks
NCH = N // 512    # 4 n-chunks of 512
MT = N // P       # 16 m-tiles of 128
DB = DC // P      # 4 d'-blocks / c-blocks


def build_program(debug=False):
    nc = bacc.Bacc("TRN2", target_bir_lowering=False, debug=False,
                   enable_asserts=False, num_devices=8)

    xT = nc.dram_tensor("xT", [D, N], dt.float32, kind="ExternalInput")
    wqT = nc.dram_tensor("wqT", [D, DC], dt.float32, kind="ExternalInput")
    wkT = nc.dram_tensor("wkT", [D, DC], dt.float32, kind="ExternalInput")
    wvT = nc.dram_tensor("wvT", [D, DC], dt.float32, kind="ExternalInput")
    wpT = nc.dram_tensor("wpT", [DC, D], dt.float32, kind="ExternalInput")
    out = nc.dram_tensor("out", [N, D], dt.float32, kind="ExternalOutput")

    f32r = dt.float32r
    f32 = dt.float32
    Exp = mybir.ActivationFunctionType.Exp
    MULT = mybir.AluOpType.mult

    with tile.TileContext(nc) as tc:
        with tc.tile_pool(name="persist", bufs=1) as persist, \
             tc.tile_pool(name="wq", bufs=1) as wq_pool, \
             tc.tile_pool(name="qTc", bufs=2) as qT_pool, \
             tc.tile_pool(name="xw", bufs=2) as xw_pool, \
             tc.tile_pool(name="ps_mm", bufs=2, space="PSUM") as ps_mm, \
             tc.tile_pool(name="ps_S", bufs=1, space="PSUM") as ps_S, \
             tc.tile_pool(name="ps_o", bufs=1, space="PSUM") as ps_o:

            # ---- persistent SBUF tensors ----
            wp_sb = persist.tile([P, DB, D], f32r, tag="wp")
            kT_sb = persist.tile([P, DB, N], f32r, tag="kT")
            # v with a ones column per head: [m-part, m-tile, head, 65]
            v_sb = persist.tile([P, MT, NHC, HD + 1], f32r, tag="v")
            ones_sb = persist.tile([P, HD], f32r, tag="ones")

            wq_sb = wq_pool.tile([P, KB, DC], f32r, tag="wq")
            nc.sync.dma_start(
                wq_sb[:], wqT.ap().rearrange("(kb p) d -> p kb d", p=P).bitcast(f32r))
            nc.sync.dma_start(
                wp_sb[:], wpT.ap().rearrange("(cb p) e -> p cb e", p=P).bitcast(f32r))
            nc.vector.memset(v_sb[:].bitcast(f32), 1.0)
            nc.vector.memset(ones_sb[:].bitcast(f32), 1.0)

            def load_xw(j, label):
                xw = xw_pool.tile([P, KB, 512], f32r, tag="xw",
                                  name=f"xw_{label}")
                nc.sync.dma_start(
                    xw[:],
                    xT.ap()[:, j * 512:(j + 1) * 512]
                    .rearrange("(kb p) n -> p kb n", p=P).bitcast(f32r))
                return xw

            def emit_proj_tiles(xw, w_sb, dst_fn, lbl):
                """q/k projection for one 512-window: 4 d'-blocks."""
                for db in range(DB):
                    pq = ps_mm.tile([P, 512], f32, tag="mm",
                                    name=f"pq_{lbl}_{db}")
                    for kb in range(KB):
                        nc.tensor.matmul(
                            pq[:],
                            lhsT=w_sb[:, kb, db * P:(db + 1) * P],
                            rhs=xw[:, kb, :],
                            start=(kb == 0), stop=(kb == KB - 1))
                    nc.vector.tensor_copy(out=dst_fn(db), in_=pq[:])

            def emit_v_window(xw, w, wv_sb):
                """v for the 4 m-tiles of window w."""
                for mc in range(4):
                    m = w * 4 + mc
                    pv = ps_mm.tile([P, 512], f32, tag="mm", name=f"pv{m}")
                    for kb in range(KB):
                        nc.tensor.matmul(
                            pv[:],
                            lhsT=xw[:, kb, mc * P:(mc + 1) * P],
                            rhs=wv_sb[:, kb, :],
                            start=(kb == 0), stop=(kb == KB - 1))
                    nc.vector.tensor_copy(
                        out=v_sb[:, m, :, 0:HD],
                        in_=pv[:].rearrange("p (h d) -> p h d", h=NHC))

            qT_tiles = [None] * NCH

            def emit_qT_chunk(j):
                qt = qT_pool.tile([P, DB, 512], f32r, tag="qTc", name=f"qT{j}")
                xwq = load_xw(j, f"q{j}")
                emit_proj_tiles(xwq, wq_sb, lambda db: qt[:, db, :], f"q{j}")
                qT_tiles[j] = qt

            # ---- prelude: kT + v (per window), then qT chunk 0 ----
            wkv_scope = tc.tile_pool(name="wkv", bufs=1)
            wkv_pool = wkv_scope.__enter__()
            wk_sb = wkv_pool.tile([P, KB, DC], f32r, tag="wk")
            wv_sb = wkv_pool.tile([P, KB, DC], f32r, tag="wv")
            nc.sync.dma_start(
                wk_sb[:], wkT.ap().rearrange("(kb p) d -> p kb d", p=P).bitcast(f32r))
            nc.sync.dma_start(
                wv_sb[:], wvT.ap().rearrange("(kb p) d -> p kb d", p=P).bitcast(f32r))

            for w in range(NCH):
                xw = load_xw(w, f"kv{w}")
                emit_proj_tiles(
                    xw, wk_sb,
                    lambda db, w=w: kT_sb[:, db, w * 512:(w + 1) * 512],
                    f"k{w}")
                emit_v_window(xw, w, wv_sb)
            emit_qT_chunk(0)

            wkv_scope.__exit__(None, None, None)

            if debug:
                dbg_qT = nc.dram_tensor("dbg_qT", [P, DB, 512], f32, kind="ExternalOutput")
                dbg_kT = nc.dram_tensor("dbg_kT", [P, DB, N], f32, kind="ExternalOutput")
                dbg_v = nc.dram_tensor("dbg_v", [P, MT, NHC, HD + 1], f32, kind="ExternalOutput")
                dbg_at = nc.dram_tensor("dbg_at", [P, DB, 512], f32, kind="ExternalOutput")
                nc.sync.dma_start(dbg_qT.ap(), qT_tiles[0][:].bitcast(f32))

            # ---- attention + projection, per n-chunk ----
            expS_scope = tc.tile_pool(name="expS", bufs=2)
            expS_pool = expS_scope.__enter__()
            at_scope = tc.tile_pool(name="at", bufs=2)
            at_pool = at_scope.__enter__()
            small_scope = tc.tile_pool(name="small", bufs=2)
            small_pool = small_scope.__enter__()
            out_scope = tc.tile_pool(name="outsb", bufs=2)
            out_pool = out_scope.__enter__()

            for j in range(NCH):
                qt = qT_tiles[j]
                at = at_pool.tile([P, DB, 512], f32r, tag="at", name=f"at{j}")

                if j + 1 < NCH:
                    # qT for the next chunk (overlaps with this chunk's attn)
                    emit_qT_chunk(j + 1)

                for p in range(DB):  # head pair p -> heads 2p, 2p+1
                    po = [ps_o.tile([P, 512], f32, tag=f"o{h}", name=f"po{h}")
                          for h in range(2)]
                    eS_prev = [None, None]

                    for mp in range(MT // 2):
                        eS_cur = [None, None]
                        for h in range(2):
                            rsl = slice(h * HD, (h + 1) * HD)
                            # attn@v of iteration mp-1 first: keeps PE ahead
                            # of ACT without blocking the next exp
                            if eS_prev[h] is not None:
                                for half in range(2):
                                    m = 2 * (mp - 1) + half
                                    nc.tensor.matmul(
                                        po[h][0:HD + 1, :],
                                        lhsT=v_sb[:, m, 2 * p + h, :],
                                        rhs=eS_prev[h][:, half * 512:(half + 1) * 512],
                                        start=(m == 0), stop=False)
                            S = ps_S.tile([P, 1024], f32, tag=f"S{h}",
                                          name=f"S{h}_{mp}")
                            for half in range(2):
                                m = 2 * mp + half
                                nc.tensor.matmul(
                                    S[:, half * 512:(half + 1) * 512],
                                    lhsT=kT_sb[rsl, p, m * P:(m + 1) * P],
                                    rhs=qt[rsl, p, :],
                                    start=True, stop=True)
                            eS = expS_pool.tile([P, 1024], f32r, tag=f"e{h}",
                                                name=f"eS{h}_{mp}")
                            nc.scalar.activation(eS[:], S[:], Exp, scale=SCALE)
                            eS_cur[h] = eS
                        eS_prev = eS_cur

                    # drain: attn@v of the last m-pair
                    for h in range(2):
                        for half in range(2):
                            m = MT - 2 + half
                            nc.tensor.matmul(
                                po[h][0:HD + 1, :],
                                lhsT=v_sb[:, m, 2 * p + h, :],
                                rhs=eS_prev[h][:, half * 512:(half + 1) * 512],
                                start=False, stop=(m == MT - 1))

                    # epilogue: copy accumulators out of PSUM immediately,
                    # normalize off the critical path.
                    for h in range(2):
                        oT = small_pool.tile([HD + 1, 512], f32, tag=f"oT{h}",
                                             name=f"oT{h}")
                        nc.vector.tensor_copy(out=oT[:], in_=po[h][0:HD + 1, :])
                        rcp = small_pool.tile([HD + 1, 512], f32r, tag="rcp",
                                              name="rcp")
                        with nc.allow_low_precision(reason="fp32r recip for PE broadcast"):
                            nc.vector.reciprocal(rcp[HD:HD + 1, :],
                                                 oT[HD:HD + 1, :])
                        bcp = ps_mm.tile([P, 512], f32, tag="mm", name=f"bcp{h}")
                        nc.tensor.matmul(bcp[0:HD, :],
                                         lhsT=ones_sb[HD:HD + 1, :],
                                         rhs=rcp[HD:HD + 1, :],
                                         start=True, stop=True)
                        if h == 0:
                            nc.vector.tensor_tensor(
                                out=at[0:HD, p, :], in0=oT[0:HD, :],
                                in1=bcp[0:HD, :], op=MULT)
                        else:
                            tmp = small_pool.tile([HD, 512], f32r, tag="tmp",
                                                  name="tmp")
                            nc.vector.tensor_tensor(
                                out=tmp[:], in0=oT[0:HD, :],
                                in1=bcp[0:HD, :], op=MULT)
                            nc.sync.dma_start(at[HD:P, p, :], tmp[:])

                if debug and j == 0:
                    nc.sync.dma_start(dbg_at.ap(), at[:].bitcast(f32))

                # projection for this n-chunk (partial over this core's 512 c)
                for ns in range(4):
                    rsl = slice(j * 512 + ns * P, j * 512 + (ns + 1) * P)
                    for ec in range(2):
                        esl = slice(ec * 512, (ec + 1) * 512)
                        pp = ps_mm.tile([P, 512], f32, tag="mm",
                                        name=f"pp{ns}_{ec}")
                        for cb in range(DB):
                            nc.tensor.matmul(
                                pp[:],
                                lhsT=at[:, cb, ns * P:(ns + 1) * P],
                                rhs=wp_sb[:, cb, esl],
                                start=(cb == 0), stop=(cb == DB - 1))
                        osb = out_pool.tile([P, 512], f32, tag="osb",
                                            name=f"osb{ns}_{ec}")
                        nc.vector.tensor_copy(out=osb[:], in_=pp[:])
                        nc.sync.dma_start(out.ap()[rsl, esl], osb[:])

            if debug:
                nc.sync.dma_start(dbg_kT.ap(), kT_sb[:].bitcast(f32))
                nc.sync.dma_start(dbg_v.ap(), v_sb[:].bitcast(f32))

            out_scope.__exit__(None, None, None)
            small_scope.__exit__(None, None, None)
            at_scope.__exit__(None, None, None)
            expS_scope.__exit__(None, None, None)

    nc.compile()
    return nc


_CACHE: dict = {}


def _get_program():
    if "nc" not in _CACHE:
        _CACHE["nc"] = build_program()
    return _CACHE["nc"]


def make_in_maps(x, w_qkv, w_proj):
    """Host-side sharding: per-core input dict."""
    x = np.ascontiguousarray(np.asarray(x, dtype=np.float32))
    w_qkv = np.asarray(w_qkv, dtype=np.float32)
    w_proj = np.asarray(w_proj, dtype=np.float32)
    in_maps = []
    for core in range(8):
        b, g = divmod(core, 2)
        gsl = slice(g * DC, (g + 1) * DC)
        in_maps.append({
            "xT": np.ascontiguousarray(x[b].T),                       # [D, N]
            "wqT": np.ascontiguousarray(w_qkv[0 * D:1 * D][gsl].T),   # [D, DC]
            "wkT": np.ascontiguousarray(w_qkv[1 * D:2 * D][gsl].T),
            "wvT": np.ascontiguousarray(w_qkv[2 * D:3 * D][gsl].T),
            "wpT": np.ascontiguousarray(w_proj[:, gsl].T),            # [DC, D]
        })
    return in_maps


def run(x, w_qkv, w_proj, b_proj, **spmd_kwargs):
    nc = _get_program()
    in_maps = make_in_maps(x, w_qkv, w_proj)
    res = run_bass_kernel_spmd(nc, in_maps, list(range(8)), **spmd_kwargs)
    b_proj = np.asarray(b_proj, dtype=np.float32)
    outp = np.empty((B, N, D), dtype=np.float32)
    for b in range(B):
        outp[b] = (res.results[2 * b]["out"] + res.results[2 * b + 1]["out"]
                   + b_proj[None, :])
    return outp, res


def kernel(x, w_qkv, w_proj, b_proj):
    outp, _ = run(x, w_qkv, w_proj, b_proj)
    return outp


# revision 15
# speedup vs baseline: 1.0359x; 1.0205x over previous
"""Multi-head attention (B=4, N=2048, D=1024, H=16) on 8 Trainium2 NeuronCores.

Sharding: core = (batch b = core//2, head-group g = core%2 of 8 heads).
Each core computes qkv + attention for its 8 heads and a *partial* output
projection over its 512 features; the host sums the two partials per batch
and adds the bias (the tensor-parallel unshard).

All matmuls run in fp32r (TF32-like, full PE speed at moving dim >=256).
Scores are computed transposed (S^T[m,n]: keys on partitions) so softmax
needs no on-chip transpose; a ones-column appended to v yields the softmax
denominators inside the same PE accumulation as attn@v.

Emission is software-pipelined for the ACT engine (exp is the per-core
roofline: 33.5M elements at 1 elem/lane/cycle): a short prelude computes
kT, v and qT(chunk 0); per m-pair the attn@v matmuls of iteration i-1 are
emitted between the score matmuls so the PE never blocks the next exp, and
qT for chunk j+1 is produced during chunk j's attention.
"""
import sys

sys.path.insert(0, '/opt/trn_rl_repo')

import numpy as np

import concourse.bass as bass  # noqa: F401  (registers engines)
import concourse.mybir as mybir
import concourse.tile as tile
from concourse import bacc
from concourse.bass_utils import run_bass_kernel_spmd

dt = mybir.dt

B = 4
N = 2048          # sequence length
D = 1024          # d_model
NH = 16           # total heads
HD = 64           # head dim
NHC = 8           # heads per core
DC = NHC * HD     # 512 features per core
SCALE = HD ** -0.5

P = 128           # partitions
KB = D // P       # 8 k-blocks
NCH = N // 512    # 4 n-chunks of 512
MT = N // P       # 16 m-tiles of 128
DB = DC // P      # 4 d'-blocks / c-blocks


def build_program(debug=False):
    nc = bacc.Bacc("TRN2", target_bir_lowering=False, debug=False,
                   enable_asserts=False, num_devices=8)

    xT = nc.dram_tensor("xT", [D, N], dt.float32, kind="ExternalInput")
    wqT = nc.dram_tensor("wqT", [D, DC], dt.float32, kind="ExternalInput")
    wkT = nc.dram_tensor("wkT", [D, DC], dt.float32, kind="ExternalInput")
    wvT = nc.dram_tensor("wvT", [D, DC], dt.float32, kind="ExternalInput")
    wpT = nc.dram_tensor("wpT", [DC, D], dt.float32, kind="ExternalInput")
    out = nc.dram_tensor("out", [N, D], dt.float32, kind="ExternalOutput")

    f32r = dt.float32r
    f32 = dt.float32
    Exp = mybir.ActivationFunctionType.Exp
    MULT = mybir.AluOpType.mult
    DIV = mybir.AluOpType.divide

    with tile.TileContext(nc) as tc:
        with tc.tile_pool(name="persist", bufs=1) as persist, \
             tc.tile_pool(name="wq", bufs=1) as wq_pool, \
             tc.tile_pool(name="qTc", bufs=2) as qT_pool, \
             tc.tile_pool(name="xw", bufs=2) as xw_pool, \
             tc.tile_pool(name="ps_mm", bufs=2, space="PSUM") as ps_mm, \
             tc.tile_pool(name="ps_S", bufs=1, space="PSUM") as ps_S, \
             tc.tile_pool(name="ps_o", bufs=1, space="PSUM") as ps_o:

            # ---- persistent SBUF tensors ----
            wp_sb = persist.tile([P, DB, D], f32r, tag="wp")
            kT_sb = persist.tile([P, DB, N], f32r, tag="kT")
            # v with a ones column per head: [m-part, m-tile, head, 65]
            v_sb = persist.tile([P, MT, NHC, HD + 1], f32r, tag="v")
            ones_sb = persist.tile([P, HD], f32r, tag="ones")

            wq_sb = wq_pool.tile([P, KB, DC], f32r, tag="wq")
            nc.sync.dma_start(
                wq_sb[:], wqT.ap().rearrange("(kb p) d -> p kb d", p=P).bitcast(f32r))
            nc.sync.dma_start(
                wp_sb[:], wpT.ap().rearrange("(cb p) e -> p cb e", p=P).bitcast(f32r))
            nc.vector.memset(v_sb[:].bitcast(f32), 1.0)
            nc.vector.memset(ones_sb[:].bitcast(f32), 1.0)

            def load_xw(j, label):
                xw = xw_pool.tile([P, KB, 512], f32r, tag="xw",
                                  name=f"xw_{label}")
                nc.sync.dma_start(
                    xw[:],
                    xT.ap()[:, j * 512:(j + 1) * 512]
                    .rearrange("(kb p) n -> p kb n", p=P).bitcast(f32r))
                return xw

            def emit_proj_tiles(xw, w_sb, dst_fn, lbl):
                """q/k projection for one 512-window: 4 d'-blocks."""
                for db in range(DB):
                    pq = ps_mm.tile([P, 512], f32, tag="mm",
                                    name=f"pq_{lbl}_{db}")
                    for kb in range(KB):
                        nc.tensor.matmul(
                            pq[:],
                            lhsT=w_sb[:, kb, db * P:(db + 1) * P],
                            rhs=xw[:, kb, :],
                            start=(kb == 0), stop=(kb == KB - 1))
                    nc.vector.tensor_copy(out=dst_fn(db), in_=pq[:])

            def emit_v_window(xw, w, wv_sb):
                """v for the 4 m-tiles of window w."""
                for mc in range(4):
                    m = w * 4 + mc
                    pv = ps_mm.tile([P, 512], f32, tag="mm", name=f"pv{m}")
                    for kb in range(KB):
                        nc.tensor.matmul(
                            pv[:],
                            lhsT=xw[:, kb, mc * P:(mc + 1) * P],
                            rhs=wv_sb[:, kb, :],
                            start=(kb == 0), stop=(kb == KB - 1))
                    nc.vector.tensor_copy(
                        out=v_sb[:, m, :, 0:HD],
                        in_=pv[:].rearrange("p (h d) -> p h d", h=NHC))

            qT_tiles = [None] * NCH

            def emit_qT_chunk(j):
                qt = qT_pool.tile([P, DB, 512], f32r, tag="qTc", name=f"qT{j}")
                xwq = load_xw(j, f"q{j}")
                emit_proj_tiles(xwq, wq_sb, lambda db: qt[:, db, :], f"q{j}")
                qT_tiles[j] = qt

            # ---- prelude: kT + v (per window), then qT chunk 0 ----
            wkv_scope = tc.tile_pool(name="wkv", bufs=1)
            wkv_pool = wkv_scope.__enter__()
            wk_sb = wkv_pool.tile([P, KB, DC], f32r, tag="wk")
            wv_sb = wkv_pool.tile([P, KB, DC], f32r, tag="wv")
            nc.sync.dma_start(
                wk_sb[:], wkT.ap().rearrange("(kb p) d -> p kb d", p=P).bitcast(f32r))
            nc.sync.dma_start(
                wv_sb[:], wvT.ap().rearrange("(kb p) d -> p kb d", p=P).bitcast(f32r))

            for w in range(NCH):
                xw = load_xw(w, f"kv{w}")
                emit_proj_tiles(
                    xw, wk_sb,
                    lambda db, w=w: kT_sb[:, db, w * 512:(w + 1) * 512],
                    f"k{w}")
                emit_v_window(xw, w, wv_sb)
            emit_qT_chunk(0)

            wkv_scope.__exit__(None, None, None)

            if debug:
                dbg_qT = nc.dram_tensor("dbg_qT", [P, DB, 512], f32, kind="ExternalOutput")
                dbg_kT = nc.dram_tensor("dbg_kT", [P, DB, N], f32, kind="ExternalOutput")
                dbg_v = nc.dram_tensor("dbg_v", [P, MT, NHC, HD + 1], f32, kind="ExternalOutput")
                dbg_at = nc.dram_tensor("dbg_at", [P, DB, 512], f32, kind="ExternalOutput")
                nc.sync.dma_start(dbg_qT.ap(), qT_tiles[0][:].bitcast(f32))

            # ---- attention + projection, per n-chunk ----
            expS_scope = tc.tile_pool(name="expS", bufs=2)
            expS_pool = expS_scope.__enter__()
            at_scope = tc.tile_pool(name="at", bufs=2)
            at_pool = at_scope.__enter__()
            small_scope = tc.tile_pool(name="small", bufs=2)
            small_pool = small_scope.__enter__()
            out_scope = tc.tile_pool(name="outsb", bufs=2)
            out_pool = out_scope.__enter__()

            for j in range(NCH):
                qt = qT_tiles[j]
                at = at_pool.tile([P, DB, 512], f32r, tag="at", name=f"at{j}")

                if j + 1 < NCH:
                    # qT for the next chunk (overlaps with this chunk's attn)
                    emit_qT_chunk(j + 1)

                for p in range(DB):  # head pair p -> heads 2p, 2p+1
                    po = [ps_o.tile([P, 512], f32, tag=f"o{h}", name=f"po{h}")
                          for h in range(2)]
                    eS_prev = [None, None]

                    for mp in range(MT // 2):
                        eS_cur = [None, None]
                        for h in range(2):
                            rsl = slice(h * HD, (h + 1) * HD)
                            # S of iteration mp first (it stalls on the WAR
                            # with exp(mp-1)); the ready attn@v of mp-1
                            # follows so the other head's chain isn't
                            # blocked long behind it.
                            S = ps_S.tile([P, 1024], f32, tag=f"S{h}",
                                          name=f"S{h}_{mp}")
                            for half in range(2):
                                m = 2 * mp + half
                                nc.tensor.matmul(
                                    S[:, half * 512:(half + 1) * 512],
                                    lhsT=kT_sb[rsl, p, m * P:(m + 1) * P],
                                    rhs=qt[rsl, p, :],
                                    start=True, stop=True)
                            eS = expS_pool.tile([P, 1024], f32r, tag=f"e{h}",
                                                name=f"eS{h}_{mp}")
                            nc.scalar.activation(eS[:], S[:], Exp, scale=SCALE)
                            eS_cur[h] = eS
                            if eS_prev[h] is not None:
                                for half in range(2):
                                    m = 2 * (mp - 1) + half
                                    nc.tensor.matmul(
                                        po[h][0:HD + 1, :],
                                        lhsT=v_sb[:, m, 2 * p + h, :],
                                        rhs=eS_prev[h][:, half * 512:(half + 1) * 512],
                                        start=(m == 0), stop=False)
                        eS_prev = eS_cur

                    # drain: attn@v of the last m-pair
                    for h in range(2):
                        for half in range(2):
                            m = MT - 2 + half
                            nc.tensor.matmul(
                                po[h][0:HD + 1, :],
                                lhsT=v_sb[:, m, 2 * p + h, :],
                                rhs=eS_prev[h][:, half * 512:(half + 1) * 512],
                                start=False, stop=(m == MT - 1))

                    # epilogue: copy accumulators out of PSUM immediately,
                    # normalize off the critical path.
                    for h in range(2):
                        oT = small_pool.tile([HD + 1, 512], f32, tag=f"oT{h}",
                                             name=f"oT{h}")
                        nc.vector.tensor_copy(out=oT[:], in_=po[h][0:HD + 1, :])
                        rf = small_pool.tile([HD + 1, 512], f32, tag="rf",
                                             name="rf")
                        nc.vector.reciprocal(rf[HD:HD + 1, :],
                                             oT[HD:HD + 1, :])
                        rcp = small_pool.tile([HD + 1, 512], f32r, tag="rcp",
                                              name="rcp")
                        nc.vector.tensor_copy(out=rcp[HD:HD + 1, :],
                                              in_=rf[HD:HD + 1, :])
                        # broadcast the reciprocal across 64 partitions via a
                        # K=1 matmul, then one DVE multiply normalizes
                        bcp = ps_mm.tile([P, 512], f32, tag="mm", name=f"bcp{h}")
                        nc.tensor.matmul(bcp[0:HD, :],
                                         lhsT=ones_sb[HD:HD + 1, :],
                                         rhs=rcp[HD:HD + 1, :],
                                         start=True, stop=True)
                        if h == 0:
                            nc.vector.tensor_tensor(
                                out=at[0:HD, p, :], in0=oT[0:HD, :],
                                in1=bcp[0:HD, :], op=MULT)
                        else:
                            tmp = small_pool.tile([HD, 512], f32r, tag="tmp",
                                                  name="tmp")
                            nc.vector.tensor_tensor(
                                out=tmp[:], in0=oT[0:HD, :],
                                in1=bcp[0:HD, :], op=MULT)
                            nc.sync.dma_start(at[HD:P, p, :], tmp[:])

                if debug and j == 0:
                    nc.sync.dma_start(dbg_at.ap(), at[:].bitcast(f32))

                # projection for this n-chunk (partial over this core's 512 c)
                for ns in range(4):
                    rsl = slice(j * 512 + ns * P, j * 512 + (ns + 1) * P)
                    for ec in range(2):
                        esl = slice(ec * 512, (ec + 1) * 512)
                        pp = ps_mm.tile([P, 512], f32, tag="mm",
                                        name=f"pp{ns}_{ec}")
                        for cb in range(DB):
                            nc.tensor.matmul(
                                pp[:],
                                lhsT=at[:, cb, ns * P:(ns + 1) * P],
                                rhs=wp_sb[:, cb, esl],
                                start=(cb == 0), stop=(cb == DB - 1))
                        osb = out_pool.tile([P, 512], f32, tag="osb",
                                            name=f"osb{ns}_{ec}")
                        nc.vector.tensor_copy(out=osb[:], in_=pp[:])
                        nc.sync.dma_start(out.ap()[rsl, esl], osb[:])

            if debug:
                nc.sync.dma_start(dbg_kT.ap(), kT_sb[:].bitcast(f32))
                nc.sync.dma_start(dbg_v.ap(), v_sb[:].bitcast(f32))

            out_scope.__exit__(None, None, None)
            small_scope.__exit__(None, None, None)
            at_scope.__exit__(None, None, None)
            expS_scope.__exit__(None, None, None)

    nc.compile()
    return nc


_CACHE: dict = {}


def _get_program():
    if "nc" not in _CACHE:
        _CACHE["nc"] = build_program()
    return _CACHE["nc"]


def make_in_maps(x, w_qkv, w_proj):
    """Host-side sharding: per-core input dict."""
    x = np.ascontiguousarray(np.asarray(x, dtype=np.float32))
    w_qkv = np.asarray(w_qkv, dtype=np.float32)
    w_proj = np.asarray(w_proj, dtype=np.float32)
    in_maps = []
    for core in range(8):
        b, g = divmod(core, 2)
        gsl = slice(g * DC, (g + 1) * DC)
        in_maps.append({
            "xT": np.ascontiguousarray(x[b].T),                       # [D, N]
            "wqT": np.ascontiguousarray(w_qkv[0 * D:1 * D][gsl].T),   # [D, DC]
            "wkT": np.ascontiguousarray(w_qkv[1 * D:2 * D][gsl].T),
            "wvT": np.ascontiguousarray(w_qkv[2 * D:3 * D][gsl].T),
            "wpT": np.ascontiguousarray(w_proj[:, gsl].T),            # [DC, D]
        })
    return in_maps


def run(x, w_qkv, w_proj, b_proj, **spmd_kwargs):
    nc = _get_program()
    in_maps = make_in_maps(x, w_qkv, w_proj)
    res = run_bass_kernel_spmd(nc, in_maps, list(range(8)), **spmd_kwargs)
    b_proj = np.asarray(b_proj, dtype=np.float32)
    outp = np.empty((B, N, D), dtype=np.float32)
    for b in range(B):
        outp[b] = (res.results[2 * b]["out"] + res.results[2 * b + 1]["out"]
                   + b_proj[None, :])
    return outp, res


def kernel(x, w_qkv, w_proj, b_proj):
    outp, _ = run(x, w_qkv, w_proj, b_proj)
    return outp
